# revision 23
# baseline (speedup 1.0000x reference)
"""Trainium2 kernel for nn_CenterDisc (segment_reduce).

Computes: per-class (4 classes) mean of x rows (N=4096 rows of 64x512),
then mean pairwise Frobenius distance between the 4 class centers.

Strategy (data-parallel over N, 8 cores):
  - host: cast x to fp8_e4m3 (the final scalar averages ~1024 rows per
    class and 32768 feature dims, so per-element quantization error
    washes out: measured rel_err ~3e-4 vs the 2e-2 gate) -> HBM traffic
    drops 4x to 16 MB/core. Shard rows 512/core, pre-arrange each shard
    into block-contiguous DMA blocks.
  - device: per-class partial sums via TensorE matmul with 4-way PE
    COLUMN TILING (stationary onehot is [128,4] -> tile_size (128,32),
    tiles at PSUM partition groups 0/32/64/96 stream 4 moving operands
    concurrently, 4 cols/cycle). Accumulate over the 4 row-chunks in
    PSUM, evict [128,gw] per group with one DVE copy, out-DMAs on the
    scalar ring. Tail blocks taper so little compute trails the stream.
  - host: add the 8 partial (4, 32768) sums, counts = bincount(labels),
    centers + pairwise norms (tiny) on host.
"""

import numpy as np

import concourse.bass as bass
import concourse.tile as tile
from concourse import bacc, mybir
from concourse.bass_utils import run_bass_kernel_spmd

# Problem shape (hardcoded per contract)
N, C, PDIM = 4096, 64, 512
D = C * PDIM           # 32768 features per row
NCLS = 4               # num classes
CORES = 8
R = N // CORES         # 512 rows per core
KP = 128               # rows per matmul chunk (partition dim)
KC = R // KP           # 4 k-chunks per core
NT = 4                 # PE column tiles (128x32 mode)
GW = 512               # moving free dim per matmul (one PSUM bank)
D4 = D // NT           # tile-local output cols per PE tile
FW = 1024              # flush width (tile-local cols per flush DMA)
NR = 8                 # one-hot replicas per PE tile quadrant

# Output-column widths per DMA block (bytes per block = W * 512).
# 3 x 4MB, then tapered tail: 2MB, 1MB, 0.5MB, 0.5MB.
WIDTHS = [8192] * 3 + [4096, 2048, 1024, 1024]
assert sum(WIDTHS) == D

_NC_CACHE = None


def _build_bass():
    nc = bacc.Bacc()
    dt8 = mybir.dt.float8e4
    x_in = nc.dram_tensor("x", [R * D], dt8, kind="ExternalInput")
    # blocked one-hot, replicated NR x: col k*(NR*NCLS) + r*NCLS + c =
    # (labels[k*KP + p] == c). The replicas fill each PSUM quadrant with
    # 8 copies of the sums so flush DMAs can rotate partitions (spreads
    # the flush reads over 8 SDMA engines instead of 2).
    oh_in = nc.dram_tensor("onehot", [KP, KC * NR * NCLS], dt8,
                           kind="ExternalInput")
    # tile-local output: [t, c, local col]; host reassembles global cols
    out = nc.dram_tensor("sums", [NT, NCLS, D4], mybir.dt.float32,
                         kind="ExternalOutput")

    with tile.TileContext(nc) as tc:
        with (
            tc.tile_pool(name="ohp", bufs=1) as ohp,
            tc.tile_pool(name="xp", bufs=4) as xp,
            tc.tile_pool(name="stp", bufs=1) as stp,
            tc.tile_pool(name="pp", bufs=8, space="PSUM") as pp,
        ):
            oht = ohp.tile([KP, KC * NR * NCLS], dt8, tag="oh")
            nc.sync.dma_start(out=oht[:], in_=oh_in[:, :])

            # persistent staging: tile t's partial sums live at
            # partitions 32t..32t+3, cols = tile-local col space
            st = stp.tile([KP, D4], mybir.dt.float32, tag="st")

            col = 0
            off = 0
            fl_done = 0
            for bi, w_blk in enumerate(WIDTHS):
                xt = xp.tile([KP, KC * w_blk], dt8, tag="x")
                nc.sync.dma_start(
                    out=xt[:],
                    in_=x_in[off:off + KP * KC * w_blk].rearrange(
                        "(p c) -> p c", p=KP))
                off += KP * KC * w_blk
                w = w_blk // NT            # cols per PE tile
                gw = min(GW, w)            # cols per matmul / psum group
                ng = w // gw
                for g in range(ng):
                    ps = pp.tile([KP, gw], mybir.dt.float32, tag="ps",
                                 name=f"ps{bi}_{g}")
                    for k in range(KC):
                        for t in range(NT):
                            c0 = k * w_blk + t * w + g * gw
                            nc.tensor.matmul(
                                ps[32 * t:32 * (t + 1), :],
                                oht[:, k * NR * NCLS:(k + 1) * NR * NCLS],
                                xt[:, c0:c0 + gw],
                                start=(k == 0),
                                stop=(k == KC - 1),
                                tile_position=(0, 32 * t),
                            )
                    lc = col // NT + g * gw
                    nc.vector.tensor_copy(out=st[:, lc:lc + gw], in_=ps[:])
                col += w_blk
                # flush completed 2048-col tile-local spans mid-stream so
                # only the last small flush trails the final matmul
                fl_hi = (col // NT) // FW * FW
                while fl_done < fl_hi:
                    f0 = fl_done
                    rp = 4 * ((f0 // FW) % NR)   # rotate replica row
                    for t in range(NT):
                        p0 = 32 * t + rp
                        nc.scalar.dma_start(
                            out=out[t, :, f0:f0 + FW],
                            in_=st[p0:p0 + NCLS, f0:f0 + FW])
                    fl_done += FW
            # final partial span: split issues across both rings
            if fl_done < D4:
                f0 = fl_done
                fw = D4 - f0
                rp = 4 * ((f0 // FW) % NR)
                for t in range(NT):
                    p0 = 32 * t + rp
                    eng = nc.scalar if t % 2 else nc.sync
                    eng.dma_start(
                        out=out[t, :, f0:f0 + fw],
                        in_=st[p0:p0 + NCLS, f0:f0 + fw])
    nc.compile()
    return nc


def _get_nc():
    global _NC_CACHE
    if _NC_CACHE is None:
        _NC_CACHE = _build_bass()
    return _NC_CACHE


def _prearrange(xs8):
    """xs8: (R, D) fp8 core shard -> flat (R*D,) block-major layout.

    Within a DMA block of width W (cols [col, col+W)): element order is
    (p, k, t, g, j) so the [KP, KC*W] tile's col k*W + t*w + g*gw + j
    holds x[k*KP+p, col + t*w + g*gw + j].
    """
    out = np.empty(R * D, dtype=xs8.dtype)
    xk = xs8.reshape(KC, KP, D)
    col = 0
    off = 0
    for w_blk in WIDTHS:
        n = KP * KC * w_blk
        blk = xk[:, :, col:col + w_blk]        # (KC, KP, W)
        out[off:off + n] = blk.transpose(1, 0, 2).reshape(-1)
        col += w_blk
        off += n
    return out


def _run(x, labels, trace=False, **spmd_kwargs):
    dt8 = mybir.dt.np(mybir.dt.float8e4)
    x = np.asarray(x, dtype=np.float32).reshape(N, D)
    x8 = x.astype(dt8)
    labels = np.asarray(labels).astype(np.int64)
    # blocked one-hot per core: (KP, KC*NCLS), col k*NCLS+c for chunk k
    lab_k = labels.reshape(CORES, KC, KP)
    oh = (lab_k[..., None] == np.arange(NCLS)).astype(dt8)   # (CORES,KC,KP,NCLS)
    oh = np.repeat(oh[:, :, :, None, :], NR, axis=3)         # replicas
    oh_blocked = oh.transpose(0, 2, 1, 3, 4).reshape(
        CORES, KP, KC * NR * NCLS)

    in_maps = [
        {"x": _prearrange(x8[c * R:(c + 1) * R]),
         "onehot": np.ascontiguousarray(oh_blocked[c])}
        for c in range(CORES)
    ]
    nc = _get_nc()
    last_err = None
    for attempt in range(3):
        try:
            br = run_bass_kernel_spmd(nc, in_maps, core_ids=list(range(CORES)),
                                      trace=trace, **spmd_kwargs)
            break
        except Exception as e:  # transient device wedge (NRT_*) — retry
            last_err = e
            import time as _time
            _time.sleep(3.0)
    else:
        raise last_err

    sums = np.zeros((NCLS, D), dtype=np.float64)
    for r in br.results:
        tl = r["sums"].astype(np.float64)      # (NT, NCLS, D4) tile-local
        col = 0
        for w_blk in WIDTHS:
            w = w_blk // NT
            lc = col // NT
            for t in range(NT):
                sums[:, col + t * w:col + (t + 1) * w] += tl[t, :, lc:lc + w]
            col += w_blk
    counts = np.bincount(labels, minlength=NCLS).astype(np.float64)
    safe = np.maximum(counts, 1.0)
    centers = sums / safe[:, None]                         # (NCLS, D)
    diffs = centers[:, None, :] - centers[None, :, :]      # (NCLS, NCLS, D)
    norms = np.sqrt(np.sum(diffs * diffs, axis=-1))        # (NCLS, NCLS)
    iu, ju = np.triu_indices(NCLS, k=1)
    distance = np.sum(norms[iu, ju]) / len(iu)
    return np.asarray(distance, dtype=np.float32), br


def kernel(x, labels):
    result, _ = _run(x, labels, trace=False)
    return result
